# revision 14
# baseline (speedup 1.0000x reference)
"""Multi-head attention (B=2,S=2048,D=1024,H=16,A=64) on 8 trn2 NeuronCores.

Sharding: core = 4*b + g  (b = batch, g = head-group of 4 heads).
Per core, everything flows in "transposed" (feature-on-partition) layout:
  qT,kT = matmuls of Wq/Wk vs xT;  v natural; S^T per head; softmax over keys
  via exp (no max-sub; scores ~ N(0,1)) with the denominator produced by a
  ones-column appended to v; normalized attT [C=256, S] per core.

Final output projection re-shard, two selectable modes:
  mode="a2a":  8-core mesh AllToAll (shard j = my attT columns for peer j's
               seq chunk; batch-duplicated since cross-batch pairs share
               nothing), then fc_out runs on BOTH received batch stacks and
               the host keeps the right one.
  mode="host": each core computes its partial fc_out over the full sequence
               of its batch; host sums the 4 partials per batch.
"""

import numpy as np

B, S, D, H, A = 2, 2048, 1024, 16, 64
GROUPS = 4              # head groups (cores per batch)
HPG = H // GROUPS       # heads per core = 4
C = HPG * A             # channels per core = 256
N_CORES = 8
SQ = S // GROUPS        # per-core output seq chunk = 512

MODE = "a2a"            # "a2a" or "host"


def build_nc(s=S, d=D, n_cores=N_CORES, mode=MODE):
    import concourse.bass as bass
    import concourse.mybir as mybir
    import concourse.tile as tile
    from concourse import bacc

    f32 = mybir.dt.float32
    f32r = mybir.dt.float32r
    AF = mybir.ActivationFunctionType

    KD = d // 128        # d-tiles (contraction for projections)
    MC = C // 128        # c-tiles per core = 2
    NS = s // 128        # seq tiles
    QC = max(1, s // 512)  # q chunks for attention
    QW = s // QC         # q chunk width (<=512)
    sq = s // GROUPS     # out rows per core chunk
    KT_PER_ST = 2        # k-tiles packed per st/pt tile (exp batching)
    OW = min(512, d)     # output free-dim chunk
    NG = NS // KT_PER_ST # st/pt groups per (h,qc)

    def r(ap):
        return ap

    nc = bacc.Bacc(
        "TRN2", target_bir_lowering=False, debug=False,
        enable_asserts=True, num_devices=n_cores,
    )

    xT_d = nc.dram_tensor("xT", [d, s], f32, kind="ExternalInput").ap()
    wq_d = nc.dram_tensor("wq", [d, C], f32, kind="ExternalInput").ap()
    wk_d = nc.dram_tensor("wk", [d, C], f32, kind="ExternalInput").ap()
    wv_d = nc.dram_tensor("wv", [d, C], f32, kind="ExternalInput").ap()
    n_wo_rows = H * A if mode == "a2a" else C
    wo_d = nc.dram_tensor("wo", [n_wo_rows, d], f32, kind="ExternalInput").ap()
    bqs_d = nc.dram_tensor("bqs", [128, MC], f32, kind="ExternalInput").ap()
    bks_d = nc.dram_tensor("bks", [128, MC], f32, kind="ExternalInput").ap()
    bvb_d = nc.dram_tensor("bvb", [128, C], f32, kind="ExternalInput").ap()
    bob_d = nc.dram_tensor("bob", [128, d], f32, kind="ExternalInput").ap()
    if mode == "a2a":
        out0_d = nc.dram_tensor("out0", [sq, d], f32, kind="ExternalOutput").ap()
        out1_d = nc.dram_tensor("out1", [sq, d], f32, kind="ExternalOutput").ap()
    else:
        out_d = nc.dram_tensor("out", [s, d], f32, kind="ExternalOutput").ap()

    with tile.TileContext(nc) as tc:
        with tc.tile_pool(name="const", bufs=1) as cpool, \
             tc.tile_pool(name="qkv", bufs=1) as qpool, \
             tc.tile_pool(name="wop", bufs=1) as wopool, \
             tc.tile_pool(name="dram", bufs=1, space="DRAM") as dpool:

            ones_f = cpool.tile([1, A], f32, name="ones_f")
            nc.vector.memset(ones_f[:], 1.0)
            ones_sb = cpool.tile([1, A], f32r, name="ones_sb")
            nc.scalar.copy(ones_sb[:], ones_f[:])
            bq_sb = cpool.tile([128, MC], f32, name="bq_sb")
            nc.sync.dma_start(bq_sb[:], bqs_d[:, :])
            bk_sb = cpool.tile([128, MC], f32, name="bk_sb")
            nc.sync.dma_start(bk_sb[:], bks_d[:, :])
            bvb_sb = cpool.tile([128, C], f32, name="bvb_sb")
            nc.sync.dma_start(bvb_sb[:], bvb_d[:, :])
            bob_sb = cpool.tile([128, d], f32, name="bob_sb")
            nc.sync.dma_start(bob_sb[:], bob_d[:, :])

            # persistent through phase 2
            qT_sb = [qpool.tile([128, s], f32r, name=f"qT{mt}", tag=f"qT{mt}")
                     for mt in range(MC)]
            kT_sb = [qpool.tile([128, s], f32r, name=f"kT{mt}", tag=f"kT{mt}")
                     for mt in range(MC)]
            # v, padded per head with a ones column: [128, NS, HPG, A+1]
            v_sb = qpool.tile([128, NS, HPG, A + 1], f32r, name="v_sb", tag="v")
            vones_f = cpool.tile([128, NS * HPG], f32, name="vones_f")
            nc.vector.memset(vones_f[:], 1.0)
            nc.vector.tensor_copy(
                v_sb[:, :, :, A],
                vones_f[:].rearrange("p (t h) -> p t h", h=HPG))

            n_wo = n_wo_rows // 128
            wo_sb = [wopool.tile([128, d], f32r, name=f"wo{kt}", tag=f"wo{kt}")
                     for kt in range(n_wo)]
            for kt in range(n_wo):
                nc.gpsimd.dma_start(wo_sb[kt][:], wo_d[kt * 128:(kt + 1) * 128, :])

            attn_sb = [qpool.tile([128, s], f32r, name=f"attn{t}", tag=f"at{t}")
                       for t in range(MC)]

            # ---------- Phase 1: load xT + weights, project q/k/v ----------
            with tc.tile_pool(name="xTw", bufs=1) as xpool, \
                 tc.tile_pool(name="ps1", bufs=3, space="PSUM") as pp1:
                xT_sb = [xpool.tile([128, s], f32r, name=f"xT{kt}", tag=f"x{kt}")
                         for kt in range(KD)]
                for kt in range(KD):
                    nc.gpsimd.dma_start(xT_sb[kt][:],
                                        xT_d[kt * 128:(kt + 1) * 128, :])
                w_sb = {}
                for wname, wd in (("q", wq_d), ("k", wk_d), ("v", wv_d)):
                    w_sb[wname] = [
                        xpool.tile([128, C], f32r, name=f"w{wname}{kt}",
                                   tag=f"w{wname}{kt}")
                        for kt in range(KD)
                    ]
                    for kt in range(KD):
                        nc.gpsimd.dma_start(w_sb[wname][kt][:],
                                            wd[kt * 128:(kt + 1) * 128, :])

                # qT / kT:  out[c, s] = sum_d W[d, c] * xT[d, s]
                for wname, dst, bias in (("q", qT_sb, bq_sb), ("k", kT_sb, bk_sb)):
                    for mt in range(MC):
                        for qc in range(QC):
                            qs = slice(qc * QW, (qc + 1) * QW)
                            ps = pp1.tile([128, QW], f32, name="ps_qk", tag="ps_qk")
                            for kt in range(KD):
                                nc.tensor.matmul(
                                    ps[:],
                                    lhsT=r(w_sb[wname][kt][:, mt * 128:(mt + 1) * 128]),
                                    rhs=r(xT_sb[kt][:, qs]),
                                    start=(kt == 0), stop=(kt == KD - 1),
                                )
                            nc.scalar.activation(
                                dst[mt][:, qs], ps[:],
                                AF.Identity, bias=bias[:, mt:mt + 1], scale=1.0,
                            )

                # v natural: out[s_tile, c] = sum_d xT[d, s_tile] * Wv[d, c]
                for st in range(NS):
                    psv = pp1.tile([128, C], f32, name="psv", tag="psv")
                    for kt in range(KD):
                        nc.tensor.matmul(
                            psv[:],
                            lhsT=r(xT_sb[kt][:, st * 128:(st + 1) * 128]),
                            rhs=r(w_sb["v"][kt][:]),
                            start=(kt == 0), stop=(kt == KD - 1),
                        )
                    nc.vector.tensor_add(
                        v_sb[:, st, :, 0:A],
                        psv[:].rearrange("p (h a) -> p h a", a=A),
                        bvb_sb[:].rearrange("p (h a) -> p h a", a=A),
                    )

            # ---------- Phase 2: attention per (head, q-chunk) ----------
            with tc.tile_pool(name="ptp", bufs=8) as ptpool, \
                 tc.tile_pool(name="sml", bufs=4) as spool, \
                 tc.tile_pool(name="pst", bufs=2, space="PSUM") as stp, \
                 tc.tile_pool(name="pav", bufs=2, space="PSUM") as avp, \
                 tc.tile_pool(name="pbc", bufs=2, space="PSUM") as bcp:

                for h in range(HPG):
                    t, off = divmod(h, 2)
                    off *= A
                    for qc in range(QC):
                        qs = slice(qc * QW, (qc + 1) * QW)
                        pts = []
                        for ng in range(NG):
                            stt = stp.tile([128, KT_PER_ST * QW], f32,
                                           name="stt", tag="st")
                            for jj in range(KT_PER_ST):
                                kt = ng * KT_PER_ST + jj
                                nc.tensor.matmul(
                                    stt[:, jj * QW:(jj + 1) * QW],
                                    lhsT=r(kT_sb[t][off:off + A,
                                                    kt * 128:(kt + 1) * 128]),
                                    rhs=r(qT_sb[t][off:off + A, qs]),
                                    start=True, stop=True,
                                )
                            pt = ptpool.tile([128, KT_PER_ST * QW], f32r,
                                             name="pt", tag="pt")
                            nc.scalar.activation(pt[:], stt[:], AF.Exp,
                                                 scale=0.125)
                            pts.append(pt)
                        av = avp.tile([A + 1, QW], f32, name="av", tag="av")
                        for kt in range(NS):
                            ng, jj = divmod(kt, KT_PER_ST)
                            nc.tensor.matmul(
                                av[:],
                                lhsT=r(v_sb[:, kt, h, :]),
                                rhs=r(pts[ng][:, jj * QW:(jj + 1) * QW]),
                                start=(kt == 0), stop=(kt == NS - 1),
                            )
                        # normalize: attn[a, q] = av[a, q] / av[A, q]
                        rec_f = spool.tile([1, QW], f32, name="rec_f",
                                           tag="rec_f")
                        nc.vector.reciprocal(rec_f[:], av[A:A + 1, :])
                        rec = spool.tile([1, QW], f32r, name="rec", tag="rec")
                        nc.vector.tensor_copy(rec[:], rec_f[:])
                        bc = bcp.tile([A, QW], f32, name="bc", tag="bc")
                        nc.tensor.matmul(bc[:], lhsT=ones_sb[:], rhs=rec[:],
                                         start=True, stop=True)
                        bcs = spool.tile([A, QW], f32, name="bcs", tag="bcs")
                        nc.scalar.copy(bcs[:], bc[:])
                        nc.vector.tensor_mul(attn_sb[t][off:off + A, qs],
                                             av[0:A, :], bcs[:])

                if mode == "a2a":
                    # ------ Phase 3: 8-core mesh AllToAll re-shard ------
                    # shard j = 4*b' + g'  gets my attT cols for seq chunk g'
                    a2a_in = dpool.tile([n_cores, MC, 128, sq], f32,
                                        name="a2a_in")
                    a2a_out = dpool.tile([n_cores, MC, 128, sq], f32,
                                         name="a2a_out")
                    for bb in range(n_cores // GROUPS):
                        for g2 in range(GROUPS):
                            for t2 in range(MC):
                                nc.sync.dma_start(
                                    a2a_in[bb * GROUPS + g2, t2, :, :],
                                    attn_sb[t2][:, g2 * sq:(g2 + 1) * sq]
                                    .bitcast(f32))
                    nc.gpsimd.collective_compute(
                        "AllToAll", mybir.AluOpType.bypass,
                        replica_groups=[list(range(n_cores))],
                        ins=[a2a_in.opt()], outs=[a2a_out.opt()],
                    )

            if mode == "a2a":
                # ---------- Phase 4: fc_out on own seq chunk, both batches --
                with tc.tile_pool(name="fcp", bufs=1) as fcpool, \
                     tc.tile_pool(name="osb", bufs=2) as opool, \
                     tc.tile_pool(name="pso", bufs=4, space="PSUM") as pso:
                    for bb, outx_d in ((0, out0_d), (1, out1_d)):
                        fc_sb = []
                        for g2 in range(GROUPS):
                            for t2 in range(MC):
                                fct = fcpool.tile([128, sq], f32r,
                                                  name=f"fc{bb}_{g2}_{t2}",
                                                  tag=f"fc{bb}_{g2}_{t2}")
                                nc.gpsimd.dma_start(
                                    fct[:], a2a_out[bb * GROUPS + g2, t2, :, :])
                                fc_sb.append(fct)
                        for mt in range(sq // 128):
                            ob = opool.tile([128, d], f32, name="ob", tag="ob")
                            for nn in range(d // OW):
                                ns_ = slice(nn * OW, (nn + 1) * OW)
                                ps = pso.tile([128, OW], f32, name="ps_o",
                                              tag="ps_o")
                                for kt in range(len(fc_sb)):
                                    nc.tensor.matmul(
                                        ps[:],
                                        lhsT=r(fc_sb[kt][:, mt * 128:(mt + 1) * 128]),
                                        rhs=r(wo_sb[kt][:, ns_]),
                                        start=(kt == 0),
                                        stop=(kt == len(fc_sb) - 1),
                                    )
                                nc.vector.tensor_add(ob[:, ns_], ps[:],
                                                     bob_sb[:, ns_])
                            nc.sync.dma_start(
                                outx_d[mt * 128:(mt + 1) * 128, :], ob[:])
            else:
                # ------- Phase 4 (host mode): partial fc_out, full seq ------
                with tc.tile_pool(name="osb", bufs=3) as opool, \
                     tc.tile_pool(name="pso", bufs=4, space="PSUM") as pso:
                    for mt in range(NS):
                        ob = opool.tile([128, d], f32, name="ob", tag="ob")
                        for nn in range(d // OW):
                            ns_ = slice(nn * OW, (nn + 1) * OW)
                            ps = pso.tile([128, OW], f32, name="ps_o",
                                          tag="ps_o")
                            for kt in range(MC):
                                nc.tensor.matmul(
                                    ps[:],
                                    lhsT=r(attn_sb[kt][:, mt * 128:(mt + 1) * 128]),
                                    rhs=r(wo_sb_g(wo_sb, kt)[:, ns_]),
                                    start=(kt == 0), stop=(kt == MC - 1),
                                )
                            nc.vector.tensor_add(ob[:, ns_], ps[:],
                                                 bob_sb[:, ns_])
                        nc.sync.dma_start(out_d[mt * 128:(mt + 1) * 128, :],
                                          ob[:])

    nc.compile()
    return nc


def wo_sb_g(wo_sb, kt):
    # host mode: contraction is only over this core's C rows of Wo; the host
    # passes the [C, d] slice in "wo" (padded tile list indexed 0..MC-1)
    return wo_sb[kt]


def make_in_maps(x, Wq, bq, Wk, bk, Wv, bv, Wo, bo, n_cores=N_CORES, mode=MODE):
    d = x.shape[2]
    MC = C // 128
    f = np.float32
    in_maps = []
    for core in range(n_cores):
        b, g = divmod(core, GROUPS)
        cs = slice(g * C, (g + 1) * C)
        m = {
            "xT": np.ascontiguousarray(x[b].T, dtype=f),
            "wq": np.ascontiguousarray(Wq[:, cs], dtype=f),
            "wk": np.ascontiguousarray(Wk[:, cs], dtype=f),
            "wv": np.ascontiguousarray(Wv[:, cs], dtype=f),
            "bqs": np.ascontiguousarray(bq[cs].reshape(MC, 128).T, dtype=f),
            "bks": np.ascontiguousarray(bk[cs].reshape(MC, 128).T, dtype=f),
            "bvb": np.ascontiguousarray(np.broadcast_to(bv[cs], (128, C)), dtype=f),
        }
        if mode == "a2a":
            m["wo"] = np.ascontiguousarray(Wo, dtype=f)
            m["bob"] = np.ascontiguousarray(np.broadcast_to(bo, (128, d)), dtype=f)
        else:
            m["wo"] = np.ascontiguousarray(Wo[cs], dtype=f)
            bob = np.broadcast_to(bo, (128, d)).astype(f) if g == 0 else \
                np.zeros((128, d), f)
            m["bob"] = np.ascontiguousarray(bob)
        in_maps.append(m)
    return in_maps


_nc_cache = {}


def _get_nc(mode=MODE):
    key = ("nc", mode)
    if key not in _nc_cache:
        _nc_cache[key] = build_nc(mode=mode)
    return _nc_cache[key]


def assemble(results, mode=MODE):
    out = np.empty((B, S, D), np.float32)
    if mode == "a2a":
        for core in range(N_CORES):
            b, g = divmod(core, GROUPS)
            out[b, g * SQ:(g + 1) * SQ, :] = results[core][f"out{b}"]
    else:
        for b in range(B):
            acc = results[b * GROUPS]["out"].copy()
            for g in range(1, GROUPS):
                acc += results[b * GROUPS + g]["out"]
            out[b] = acc
    return out


def kernel(x, Wq, bq, Wk, bk, Wv, bv, Wo, bo, _trace=False, _mode=None):
    from concourse.bass_utils import run_bass_kernel_spmd

    mode = _mode or MODE
    nc = _get_nc(mode)
    in_maps = make_in_maps(x, Wq, bq, Wk, bk, Wv, bv, Wo, bo, mode=mode)
    res = run_bass_kernel_spmd(nc, in_maps, core_ids=list(range(N_CORES)),
                               trace=_trace)
    _nc_cache["last_result"] = res
    return assemble(res.results, mode=mode)


# revision 16
# speedup vs baseline: 1.0753x; 1.0753x over previous
"""Multi-head attention (B=2,S=2048,D=1024,H=16,A=64) on 8 trn2 NeuronCores.

Sharding: core = 4*b + g  (b = batch, g = head-group of 4 heads).
Per core, everything flows in "transposed" (feature-on-partition) layout:
  qT,kT = matmuls of Wq/Wk vs xT;  v natural; S^T per head; softmax over keys
  via exp (no max-sub; scores ~ N(0,1)) with the denominator produced by a
  ones-column appended to v; normalized attT [C=256, S] per core.

Final output projection re-shard, two selectable modes:
  mode="a2a":  8-core mesh AllToAll (shard j = my attT columns for peer j's
               seq chunk; batch-duplicated since cross-batch pairs share
               nothing), then fc_out runs on BOTH received batch stacks and
               the host keeps the right one.
  mode="host": each core computes its partial fc_out over the full sequence
               of its batch; host sums the 4 partials per batch.
"""

import numpy as np

B, S, D, H, A = 2, 2048, 1024, 16, 64
GROUPS = 4              # head groups (cores per batch)
HPG = H // GROUPS       # heads per core = 4
C = HPG * A             # channels per core = 256
N_CORES = 8
SQ = S // GROUPS        # per-core output seq chunk = 512

MODE = "a2a"            # "a2a" or "host"
CDT = "bf16"            # matmul compute dtype: "f32r" or "bf16"


def build_nc(s=S, d=D, n_cores=N_CORES, mode=MODE, cdt=None):
    import concourse.bass as bass
    import concourse.mybir as mybir
    import concourse.tile as tile
    from concourse import bacc

    f32 = mybir.dt.float32
    cdt = cdt or CDT
    f32r = mybir.dt.bfloat16 if cdt == "bf16" else mybir.dt.float32r
    AF = mybir.ActivationFunctionType

    KD = d // 128        # d-tiles (contraction for projections)
    MC = C // 128        # c-tiles per core = 2
    NS = s // 128        # seq tiles
    QC = max(1, s // 512)  # q chunks for attention
    QW = s // QC         # q chunk width (<=512)
    sq = s // GROUPS     # out rows per core chunk
    KT_PER_ST = 2        # k-tiles packed per st/pt tile (exp batching)
    OW = min(512, d)     # output free-dim chunk
    NG = NS // KT_PER_ST # st/pt groups per (h,qc)

    def r(ap):
        return ap

    nc = bacc.Bacc(
        "TRN2", target_bir_lowering=False, debug=False,
        enable_asserts=True, num_devices=n_cores,
    )

    xT_d = nc.dram_tensor("xT", [d, s], f32, kind="ExternalInput").ap()
    wq_d = nc.dram_tensor("wq", [d, C], f32, kind="ExternalInput").ap()
    wk_d = nc.dram_tensor("wk", [d, C], f32, kind="ExternalInput").ap()
    wv_d = nc.dram_tensor("wv", [d, C], f32, kind="ExternalInput").ap()
    n_wo_rows = H * A if mode == "a2a" else C
    wo_d = nc.dram_tensor("wo", [n_wo_rows, d], f32, kind="ExternalInput").ap()
    bqs_d = nc.dram_tensor("bqs", [128, MC], f32, kind="ExternalInput").ap()
    bks_d = nc.dram_tensor("bks", [128, MC], f32, kind="ExternalInput").ap()
    bvb_d = nc.dram_tensor("bvb", [128, C], f32, kind="ExternalInput").ap()
    bob_d = nc.dram_tensor("bob", [128, d], f32, kind="ExternalInput").ap()
    if mode == "a2a":
        out0_d = nc.dram_tensor("out0", [sq, d], f32, kind="ExternalOutput").ap()
        out1_d = nc.dram_tensor("out1", [sq, d], f32, kind="ExternalOutput").ap()
    else:
        out_d = nc.dram_tensor("out", [s, d], f32, kind="ExternalOutput").ap()

    with tile.TileContext(nc) as tc:
        with tc.tile_pool(name="const", bufs=1) as cpool, \
             tc.tile_pool(name="qkv", bufs=1) as qpool, \
             tc.tile_pool(name="wop", bufs=1) as wopool, \
             tc.tile_pool(name="dram", bufs=1, space="DRAM") as dpool:

            ones_f = cpool.tile([1, A], f32, name="ones_f")
            nc.vector.memset(ones_f[:], 1.0)
            ones_sb = cpool.tile([1, A], f32r, name="ones_sb")
            nc.scalar.copy(ones_sb[:], ones_f[:])
            bq_sb = cpool.tile([128, MC], f32, name="bq_sb")
            nc.sync.dma_start(bq_sb[:], bqs_d[:, :])
            bk_sb = cpool.tile([128, MC], f32, name="bk_sb")
            nc.sync.dma_start(bk_sb[:], bks_d[:, :])
            bvb_sb = cpool.tile([128, C], f32, name="bvb_sb")
            nc.sync.dma_start(bvb_sb[:], bvb_d[:, :])
            bob_sb = cpool.tile([128, d], f32, name="bob_sb")
            nc.sync.dma_start(bob_sb[:], bob_d[:, :])

            # persistent through phase 2
            qT_sb = [qpool.tile([128, s], f32r, name=f"qT{mt}", tag=f"qT{mt}")
                     for mt in range(MC)]
            kT_sb = [qpool.tile([128, s], f32r, name=f"kT{mt}", tag=f"kT{mt}")
                     for mt in range(MC)]
            # v, padded per head with a ones column: [128, NS, HPG, A+1]
            v_sb = qpool.tile([128, NS, HPG, A + 1], f32r, name="v_sb", tag="v")
            vones_f = cpool.tile([128, NS * HPG], f32, name="vones_f")
            nc.vector.memset(vones_f[:], 1.0)
            nc.vector.tensor_copy(
                v_sb[:, :, :, A],
                vones_f[:].rearrange("p (t h) -> p t h", h=HPG))

            n_wo = n_wo_rows // 128
            wo_sb = [wopool.tile([128, d], f32r, name=f"wo{kt}", tag=f"wo{kt}")
                     for kt in range(n_wo)]
            for kt in range(n_wo):
                nc.gpsimd.dma_start(wo_sb[kt][:], wo_d[kt * 128:(kt + 1) * 128, :])

            attn_sb = [qpool.tile([128, s], f32r, name=f"attn{t}", tag=f"at{t}")
                       for t in range(MC)]

            # ---------- Phase 1: load xT + weights, project q/k/v ----------
            with tc.tile_pool(name="xTw", bufs=1) as xpool, \
                 tc.tile_pool(name="ps1", bufs=3, space="PSUM") as pp1:
                xT_sb = [xpool.tile([128, s], f32r, name=f"xT{kt}", tag=f"x{kt}")
                         for kt in range(KD)]
                for kt in range(KD):
                    nc.gpsimd.dma_start(xT_sb[kt][:],
                                        xT_d[kt * 128:(kt + 1) * 128, :])
                w_sb = {}
                for wname, wd in (("q", wq_d), ("k", wk_d), ("v", wv_d)):
                    w_sb[wname] = [
                        xpool.tile([128, C], f32r, name=f"w{wname}{kt}",
                                   tag=f"w{wname}{kt}")
                        for kt in range(KD)
                    ]
                    for kt in range(KD):
                        nc.gpsimd.dma_start(w_sb[wname][kt][:],
                                            wd[kt * 128:(kt + 1) * 128, :])

                # qT / kT:  out[c, s] = sum_d W[d, c] * xT[d, s]
                for wname, dst, bias in (("q", qT_sb, bq_sb), ("k", kT_sb, bk_sb)):
                    for mt in range(MC):
                        for qc in range(QC):
                            qs = slice(qc * QW, (qc + 1) * QW)
                            ps = pp1.tile([128, QW], f32, name="ps_qk", tag="ps_qk")
                            for kt in range(KD):
                                nc.tensor.matmul(
                                    ps[:],
                                    lhsT=r(w_sb[wname][kt][:, mt * 128:(mt + 1) * 128]),
                                    rhs=r(xT_sb[kt][:, qs]),
                                    start=(kt == 0), stop=(kt == KD - 1),
                                )
                            nc.scalar.activation(
                                dst[mt][:, qs], ps[:],
                                AF.Identity, bias=bias[:, mt:mt + 1], scale=1.0,
                            )

                # v natural: out[s_tile, c] = sum_d xT[d, s_tile] * Wv[d, c]
                for st in range(NS):
                    psv = pp1.tile([128, C], f32, name="psv", tag="psv")
                    for kt in range(KD):
                        nc.tensor.matmul(
                            psv[:],
                            lhsT=r(xT_sb[kt][:, st * 128:(st + 1) * 128]),
                            rhs=r(w_sb["v"][kt][:]),
                            start=(kt == 0), stop=(kt == KD - 1),
                        )
                    nc.vector.tensor_add(
                        v_sb[:, st, :, 0:A],
                        psv[:].rearrange("p (h a) -> p h a", a=A),
                        bvb_sb[:].rearrange("p (h a) -> p h a", a=A),
                    )

            # ---------- Phase 2: attention per (head, q-chunk) ----------
            with tc.tile_pool(name="ptp", bufs=8) as ptpool, \
                 tc.tile_pool(name="sml", bufs=4) as spool, \
                 tc.tile_pool(name="pst", bufs=2, space="PSUM") as stp, \
                 tc.tile_pool(name="pav", bufs=2, space="PSUM") as avp, \
                 tc.tile_pool(name="pbc", bufs=2, space="PSUM") as bcp:

                for h in range(HPG):
                    t, off = divmod(h, 2)
                    off *= A
                    for qc in range(QC):
                        qs = slice(qc * QW, (qc + 1) * QW)
                        pts = []
                        for ng in range(NG):
                            stt = stp.tile([128, KT_PER_ST * QW], f32,
                                           name="stt", tag="st")
                            for jj in range(KT_PER_ST):
                                kt = ng * KT_PER_ST + jj
                                nc.tensor.matmul(
                                    stt[:, jj * QW:(jj + 1) * QW],
                                    lhsT=r(kT_sb[t][off:off + A,
                                                    kt * 128:(kt + 1) * 128]),
                                    rhs=r(qT_sb[t][off:off + A, qs]),
                                    start=True, stop=True,
                                )
                            pt = ptpool.tile([128, KT_PER_ST * QW], f32r,
                                             name="pt", tag="pt")
                            nc.scalar.activation(pt[:], stt[:], AF.Exp,
                                                 scale=0.125)
                            pts.append(pt)
                        av = avp.tile([A + 1, QW], f32, name="av", tag="av")
                        for kt in range(NS):
                            ng, jj = divmod(kt, KT_PER_ST)
                            nc.tensor.matmul(
                                av[:],
                                lhsT=r(v_sb[:, kt, h, :]),
                                rhs=r(pts[ng][:, jj * QW:(jj + 1) * QW]),
                                start=(kt == 0), stop=(kt == NS - 1),
                            )
                        # normalize: attn[a, q] = av[a, q] / av[A, q]
                        rec_f = spool.tile([1, QW], f32, name="rec_f",
                                           tag="rec_f")
                        nc.vector.reciprocal(rec_f[:], av[A:A + 1, :])
                        rec = spool.tile([1, QW], f32r, name="rec", tag="rec")
                        nc.vector.tensor_copy(rec[:], rec_f[:])
                        bc = bcp.tile([A, QW], f32, name="bc", tag="bc")
                        nc.tensor.matmul(bc[:], lhsT=ones_sb[:], rhs=rec[:],
                                         start=True, stop=True)
                        bcs = spool.tile([A, QW], f32, name="bcs", tag="bcs")
                        nc.scalar.copy(bcs[:], bc[:])
                        nc.vector.tensor_mul(attn_sb[t][off:off + A, qs],
                                             av[0:A, :], bcs[:])

                if mode == "a2a":
                    # ------ Phase 3: 8-core mesh AllToAll re-shard ------
                    # shard j = 4*b' + g'  gets my attT cols for seq chunk g'
                    a2a_in = dpool.tile([n_cores, MC, 128, sq], f32r,
                                        name="a2a_in")
                    a2a_out = dpool.tile([n_cores, MC, 128, sq], f32r,
                                         name="a2a_out")
                    for bb in range(n_cores // GROUPS):
                        for g2 in range(GROUPS):
                            for t2 in range(MC):
                                nc.sync.dma_start(
                                    a2a_in[bb * GROUPS + g2, t2, :, :],
                                    attn_sb[t2][:, g2 * sq:(g2 + 1) * sq])
                    nc.gpsimd.collective_compute(
                        "AllToAll", mybir.AluOpType.bypass,
                        replica_groups=[list(range(n_cores))],
                        ins=[a2a_in.opt()], outs=[a2a_out.opt()],
                    )

            if mode == "a2a":
                # ---------- Phase 4: fc_out on own seq chunk, both batches --
                with tc.tile_pool(name="fcp", bufs=1) as fcpool, \
                     tc.tile_pool(name="osb", bufs=2) as opool, \
                     tc.tile_pool(name="pso", bufs=4, space="PSUM") as pso:
                    for bb, outx_d in ((0, out0_d), (1, out1_d)):
                        fc_sb = []
                        for g2 in range(GROUPS):
                            for t2 in range(MC):
                                fct = fcpool.tile([128, sq], f32r,
                                                  name=f"fc{bb}_{g2}_{t2}",
                                                  tag=f"fc{bb}_{g2}_{t2}")
                                nc.gpsimd.dma_start(
                                    fct[:], a2a_out[bb * GROUPS + g2, t2, :, :])
                                fc_sb.append(fct)
                        for mt in range(sq // 128):
                            ob = opool.tile([128, d], f32, name="ob", tag="ob")
                            for nn in range(d // OW):
                                ns_ = slice(nn * OW, (nn + 1) * OW)
                                ps = pso.tile([128, OW], f32, name="ps_o",
                                              tag="ps_o")
                                for kt in range(len(fc_sb)):
                                    nc.tensor.matmul(
                                        ps[:],
                                        lhsT=r(fc_sb[kt][:, mt * 128:(mt + 1) * 128]),
                                        rhs=r(wo_sb[kt][:, ns_]),
                                        start=(kt == 0),
                                        stop=(kt == len(fc_sb) - 1),
                                    )
                                nc.vector.tensor_add(ob[:, ns_], ps[:],
                                                     bob_sb[:, ns_])
                            nc.sync.dma_start(
                                outx_d[mt * 128:(mt + 1) * 128, :], ob[:])
            else:
                # ------- Phase 4 (host mode): partial fc_out, full seq ------
                with tc.tile_pool(name="osb", bufs=3) as opool, \
                     tc.tile_pool(name="pso", bufs=4, space="PSUM") as pso:
                    for mt in range(NS):
                        ob = opool.tile([128, d], f32, name="ob", tag="ob")
                        for nn in range(d // OW):
                            ns_ = slice(nn * OW, (nn + 1) * OW)
                            ps = pso.tile([128, OW], f32, name="ps_o",
                                          tag="ps_o")
                            for kt in range(MC):
                                nc.tensor.matmul(
                                    ps[:],
                                    lhsT=r(attn_sb[kt][:, mt * 128:(mt + 1) * 128]),
                                    rhs=r(wo_sb_g(wo_sb, kt)[:, ns_]),
                                    start=(kt == 0), stop=(kt == MC - 1),
                                )
                            nc.vector.tensor_add(ob[:, ns_], ps[:],
                                                 bob_sb[:, ns_])
                        nc.sync.dma_start(out_d[mt * 128:(mt + 1) * 128, :],
                                          ob[:])

    nc.compile()
    return nc


def wo_sb_g(wo_sb, kt):
    # host mode: contraction is only over this core's C rows of Wo; the host
    # passes the [C, d] slice in "wo" (padded tile list indexed 0..MC-1)
    return wo_sb[kt]


def make_in_maps(x, Wq, bq, Wk, bk, Wv, bv, Wo, bo, n_cores=N_CORES, mode=MODE):
    d = x.shape[2]
    MC = C // 128
    f = np.float32
    in_maps = []
    for core in range(n_cores):
        b, g = divmod(core, GROUPS)
        cs = slice(g * C, (g + 1) * C)
        m = {
            "xT": np.ascontiguousarray(x[b].T, dtype=f),
            "wq": np.ascontiguousarray(Wq[:, cs], dtype=f),
            "wk": np.ascontiguousarray(Wk[:, cs], dtype=f),
            "wv": np.ascontiguousarray(Wv[:, cs], dtype=f),
            "bqs": np.ascontiguousarray(bq[cs].reshape(MC, 128).T, dtype=f),
            "bks": np.ascontiguousarray(bk[cs].reshape(MC, 128).T, dtype=f),
            "bvb": np.ascontiguousarray(np.broadcast_to(bv[cs], (128, C)), dtype=f),
        }
        if mode == "a2a":
            m["wo"] = np.ascontiguousarray(Wo, dtype=f)
            m["bob"] = np.ascontiguousarray(np.broadcast_to(bo, (128, d)), dtype=f)
        else:
            m["wo"] = np.ascontiguousarray(Wo[cs], dtype=f)
            bob = np.broadcast_to(bo, (128, d)).astype(f) if g == 0 else \
                np.zeros((128, d), f)
            m["bob"] = np.ascontiguousarray(bob)
        in_maps.append(m)
    return in_maps


_nc_cache = {}


def _get_nc(mode=MODE):
    key = ("nc", mode)
    if key not in _nc_cache:
        _nc_cache[key] = build_nc(mode=mode)
    return _nc_cache[key]


def assemble(results, mode=MODE):
    out = np.empty((B, S, D), np.float32)
    if mode == "a2a":
        for core in range(N_CORES):
            b, g = divmod(core, GROUPS)
            out[b, g * SQ:(g + 1) * SQ, :] = results[core][f"out{b}"]
    else:
        for b in range(B):
            acc = results[b * GROUPS]["out"].copy()
            for g in range(1, GROUPS):
                acc += results[b * GROUPS + g]["out"]
            out[b] = acc
    return out


def kernel(x, Wq, bq, Wk, bk, Wv, bv, Wo, bo, _trace=False, _mode=None):
    from concourse.bass_utils import run_bass_kernel_spmd

    mode = _mode or MODE
    nc = _get_nc(mode)
    in_maps = make_in_maps(x, Wq, bq, Wk, bk, Wv, bv, Wo, bo, mode=mode)
    res = run_bass_kernel_spmd(nc, in_maps, core_ids=list(range(N_CORES)),
                               trace=_trace)
    _nc_cache["last_result"] = res
    return assemble(res.results, mode=mode)


# revision 20
# speedup vs baseline: 1.1087x; 1.0310x over previous
"""Multi-head attention (B=2,S=2048,D=1024,H=16,A=64) on 8 trn2 NeuronCores.

Sharding: core = 4*b + g  (b = batch, g = head-group of 4 heads).
Per core, everything flows in "transposed" (feature-on-partition) layout:
  qT,kT = matmuls of Wq/Wk vs xT;  v natural; S^T per head; softmax over keys
  via exp (no max-sub; scores ~ N(0,1)) with the denominator produced by a
  ones-column appended to v; normalized attT [C=256, S] per core.

Final output projection re-shard, two selectable modes:
  mode="a2a":  8-core mesh AllToAll (shard j = my attT columns for peer j's
               seq chunk; batch-duplicated since cross-batch pairs share
               nothing), then fc_out runs on BOTH received batch stacks and
               the host keeps the right one.
  mode="host": each core computes its partial fc_out over the full sequence
               of its batch; host sums the 4 partials per batch.
"""

import numpy as np

B, S, D, H, A = 2, 2048, 1024, 16, 64
GROUPS = 4              # head groups (cores per batch)
HPG = H // GROUPS       # heads per core = 4
C = HPG * A             # channels per core = 256
N_CORES = 8
SQ = S // GROUPS        # per-core output seq chunk = 512

MODE = "a2a"            # "a2a" or "host"
CDT = "bf16"            # matmul compute dtype: "f32r" or "bf16"


def build_nc(s=S, d=D, n_cores=N_CORES, mode=MODE, cdt=None):
    import concourse.bass as bass
    import concourse.mybir as mybir
    import concourse.tile as tile
    from concourse import bacc

    f32 = mybir.dt.float32
    cdt = cdt or CDT
    f32r = mybir.dt.bfloat16 if cdt == "bf16" else mybir.dt.float32r
    AF = mybir.ActivationFunctionType

    KD = d // 128        # d-tiles (contraction for projections)
    MC = C // 128        # c-tiles per core = 2
    NS = s // 128        # seq tiles
    QC = max(1, s // 512)  # q chunks for attention
    QW = s // QC         # q chunk width (<=512)
    sq = s // GROUPS     # out rows per core chunk
    KT_PER_ST = 2        # k-tiles packed per st/pt tile (exp batching)
    OW = min(512, d)     # output free-dim chunk
    NG = NS // KT_PER_ST # st/pt groups per (h,qc)

    def r(ap):
        return ap

    nc = bacc.Bacc(
        "TRN2", target_bir_lowering=False, debug=False,
        enable_asserts=True, num_devices=n_cores,
    )

    idt = f32r if cdt == "bf16" else f32   # host pre-converts in bf16 mode
    xT_d = nc.dram_tensor("xT", [d, s], idt, kind="ExternalInput").ap()
    wq_d = nc.dram_tensor("wq", [d, C], idt, kind="ExternalInput").ap()
    wk_d = nc.dram_tensor("wk", [d, C], idt, kind="ExternalInput").ap()
    wv_d = nc.dram_tensor("wv", [d, C], idt, kind="ExternalInput").ap()
    n_wo_rows = H * A if mode == "a2a" else C
    wo_d = nc.dram_tensor("wo", [n_wo_rows, d], idt, kind="ExternalInput").ap()
    hwdge = [nc.sync, nc.scalar, nc.gpsimd]

    def load(i, dst, src_ap):
        # bf16: plain HWDGE spread across engines; f32r: gpsimd casting DMA
        if cdt == "bf16":
            hwdge[i % 3].dma_start(dst, src_ap)
        else:
            nc.gpsimd.dma_start(dst, src_ap)
    bqs_d = nc.dram_tensor("bqs", [128, MC], f32, kind="ExternalInput").ap()
    bks_d = nc.dram_tensor("bks", [128, MC], f32, kind="ExternalInput").ap()
    bvb_d = nc.dram_tensor("bvb", [128, C], f32, kind="ExternalInput").ap()
    bob_d = nc.dram_tensor("bob", [128, d], f32, kind="ExternalInput").ap()
    if mode == "a2a":
        out0_d = nc.dram_tensor("out0", [sq, d], f32, kind="ExternalOutput").ap()
        out1_d = nc.dram_tensor("out1", [sq, d], f32, kind="ExternalOutput").ap()
    else:
        out_d = nc.dram_tensor("out", [s, d], f32, kind="ExternalOutput").ap()

    with tile.TileContext(nc) as tc:
        with tc.tile_pool(name="const", bufs=1) as cpool, \
             tc.tile_pool(name="qkv", bufs=1) as qpool, \
             tc.tile_pool(name="wop", bufs=1) as wopool, \
             tc.tile_pool(name="dram", bufs=1, space="DRAM") as dpool:

            ones_f = cpool.tile([1, A], f32, name="ones_f")
            nc.vector.memset(ones_f[:], 1.0)
            ones_sb = cpool.tile([1, A], f32r, name="ones_sb")
            nc.scalar.copy(ones_sb[:], ones_f[:])
            bq_sb = cpool.tile([128, MC], f32, name="bq_sb")
            nc.sync.dma_start(bq_sb[:], bqs_d[:, :])
            bk_sb = cpool.tile([128, MC], f32, name="bk_sb")
            nc.sync.dma_start(bk_sb[:], bks_d[:, :])
            bvb_sb = cpool.tile([128, C], f32, name="bvb_sb")
            nc.sync.dma_start(bvb_sb[:], bvb_d[:, :])
            bob_sb = cpool.tile([128, d], f32, name="bob_sb")
            nc.sync.dma_start(bob_sb[:], bob_d[:, :])

            # persistent through phase 2
            qT_sb = [qpool.tile([128, s], f32r, name=f"qT{mt}", tag=f"qT{mt}")
                     for mt in range(MC)]
            kT_sb = [qpool.tile([128, s], f32r, name=f"kT{mt}", tag=f"kT{mt}")
                     for mt in range(MC)]
            # v, padded per head with a ones column: [128, NS, HPG, A+1]
            v_sb = qpool.tile([128, NS, HPG, A + 1], f32r, name="v_sb", tag="v")
            vones_f = cpool.tile([128, NS * HPG], f32, name="vones_f")
            nc.vector.memset(vones_f[:], 1.0)
            nc.vector.tensor_copy(
                v_sb[:, :, :, A],
                vones_f[:].rearrange("p (t h) -> p t h", h=HPG))

            n_wo = n_wo_rows // 128
            wo_sb = [wopool.tile([128, d], f32r, name=f"wo{kt}", tag=f"wo{kt}")
                     for kt in range(n_wo)]
            for kt in range(n_wo):
                load(kt, wo_sb[kt][:], wo_d[kt * 128:(kt + 1) * 128, :])

            attn_sb = [qpool.tile([128, s], f32r, name=f"attn{t}", tag=f"at{t}")
                       for t in range(MC)]

            # ---------- Phase 1: load xT + weights, project q/k/v ----------
            with tc.tile_pool(name="xTw", bufs=1) as xpool, \
                 tc.tile_pool(name="ps1", bufs=3, space="PSUM") as pp1:
                xT_sb = [xpool.tile([128, s], f32r, name=f"xT{kt}", tag=f"x{kt}")
                         for kt in range(KD)]
                w_sb = {}
                for wname in ("q", "k", "v"):
                    w_sb[wname] = [
                        xpool.tile([128, C], f32r, name=f"w{wname}{kt}",
                                   tag=f"w{wname}{kt}")
                        for kt in range(KD)
                    ]
                wds = {"q": wq_d, "k": wk_d, "v": wv_d}
                li = 0
                for kt in range(KD):
                    ks = slice(kt * 128, (kt + 1) * 128)
                    load(li, xT_sb[kt][:], xT_d[ks, :]); li += 1
                    for wname in ("q", "k", "v"):
                        load(li, w_sb[wname][kt][:], wds[wname][ks, :]); li += 1

                # qT / kT:  out[c, s] = sum_d W[d, c] * xT[d, s]
                for wname, dst, bias in (("q", qT_sb, bq_sb), ("k", kT_sb, bk_sb)):
                    for mt in range(MC):
                        for qc in range(QC):
                            qs = slice(qc * QW, (qc + 1) * QW)
                            ps = pp1.tile([128, QW], f32, name="ps_qk", tag="ps_qk")
                            for kt in range(KD):
                                nc.tensor.matmul(
                                    ps[:],
                                    lhsT=r(w_sb[wname][kt][:, mt * 128:(mt + 1) * 128]),
                                    rhs=r(xT_sb[kt][:, qs]),
                                    start=(kt == 0), stop=(kt == KD - 1),
                                )
                            nc.scalar.activation(
                                dst[mt][:, qs], ps[:],
                                AF.Identity, bias=bias[:, mt:mt + 1], scale=1.0,
                            )

                # v natural: out[s_tile, c] = sum_d xT[d, s_tile] * Wv[d, c]
                for st in range(NS):
                    psv = pp1.tile([128, C], f32, name="psv", tag="psv")
                    for kt in range(KD):
                        nc.tensor.matmul(
                            psv[:],
                            lhsT=r(xT_sb[kt][:, st * 128:(st + 1) * 128]),
                            rhs=r(w_sb["v"][kt][:]),
                            start=(kt == 0), stop=(kt == KD - 1),
                        )
                    nc.vector.tensor_add(
                        v_sb[:, st, :, 0:A],
                        psv[:].rearrange("p (h a) -> p h a", a=A),
                        bvb_sb[:].rearrange("p (h a) -> p h a", a=A),
                    )

            # ---------- Phase 2: attention per (head, q-chunk) ----------
            with tc.tile_pool(name="ptp", bufs=8) as ptpool, \
                 tc.tile_pool(name="sml", bufs=4) as spool, \
                 tc.tile_pool(name="pst", bufs=2, space="PSUM") as stp, \
                 tc.tile_pool(name="pav", bufs=2, space="PSUM") as avp, \
                 tc.tile_pool(name="pbc", bufs=2, space="PSUM") as bcp:

                for h in range(HPG):
                    t, off = divmod(h, 2)
                    off *= A
                    for qc in range(QC):
                        qs = slice(qc * QW, (qc + 1) * QW)
                        pts = []
                        for ng in range(NG):
                            stt = stp.tile([128, KT_PER_ST * QW], f32,
                                           name="stt", tag="st")
                            for jj in range(KT_PER_ST):
                                kt = ng * KT_PER_ST + jj
                                nc.tensor.matmul(
                                    stt[:, jj * QW:(jj + 1) * QW],
                                    lhsT=r(kT_sb[t][off:off + A,
                                                    kt * 128:(kt + 1) * 128]),
                                    rhs=r(qT_sb[t][off:off + A, qs]),
                                    start=True, stop=True,
                                )
                            pt = ptpool.tile([128, KT_PER_ST * QW], f32r,
                                             name="pt", tag="pt")
                            nc.scalar.activation(pt[:], stt[:], AF.Exp,
                                                 scale=0.125)
                            pts.append(pt)
                        av = avp.tile([A + 1, QW], f32, name="av", tag="av")
                        for kt in range(NS):
                            ng, jj = divmod(kt, KT_PER_ST)
                            nc.tensor.matmul(
                                av[:],
                                lhsT=r(v_sb[:, kt, h, :]),
                                rhs=r(pts[ng][:, jj * QW:(jj + 1) * QW]),
                                start=(kt == 0), stop=(kt == NS - 1),
                            )
                        # normalize: attn[a, q] = av[a, q] / av[A, q]
                        rec_f = spool.tile([1, QW], f32, name="rec_f",
                                           tag="rec_f")
                        nc.vector.reciprocal(rec_f[:], av[A:A + 1, :])
                        rec = spool.tile([1, QW], f32r, name="rec", tag="rec")
                        nc.vector.tensor_copy(rec[:], rec_f[:])
                        bc = bcp.tile([A, QW], f32, name="bc", tag="bc")
                        nc.tensor.matmul(bc[:], lhsT=ones_sb[:], rhs=rec[:],
                                         start=True, stop=True)
                        bcs = spool.tile([A, QW], f32, name="bcs", tag="bcs")
                        nc.scalar.copy(bcs[:], bc[:])
                        nc.vector.tensor_mul(attn_sb[t][off:off + A, qs],
                                             av[0:A, :], bcs[:])

                if mode == "a2a":
                    # ------ Phase 3: 8-core mesh AllToAll re-shard ------
                    # shard j = 4*b' + g'  gets my attT cols for seq chunk g'
                    a2a_in = dpool.tile([n_cores, MC, 128, sq], f32r,
                                        name="a2a_in")
                    a2a_out = dpool.tile([n_cores, MC, 128, sq], f32r,
                                         name="a2a_out")
                    for bb in range(n_cores // GROUPS):
                        for g2 in range(GROUPS):
                            for t2 in range(MC):
                                nc.sync.dma_start(
                                    a2a_in[bb * GROUPS + g2, t2, :, :],
                                    attn_sb[t2][:, g2 * sq:(g2 + 1) * sq])
                    nc.gpsimd.collective_compute(
                        "AllToAll", mybir.AluOpType.bypass,
                        replica_groups=[list(range(n_cores))],
                        ins=[a2a_in.opt()], outs=[a2a_out.opt()],
                    )

            if mode == "a2a":
                # ---------- Phase 4: fc_out on own seq chunk, both batches --
                with tc.tile_pool(name="fcp", bufs=1) as fcpool, \
                     tc.tile_pool(name="osb", bufs=2) as opool, \
                     tc.tile_pool(name="pso", bufs=4, space="PSUM") as pso:
                    for bb, outx_d in ((0, out0_d), (1, out1_d)):
                        fc_sb = []
                        for g2 in range(GROUPS):
                            for t2 in range(MC):
                                fct = fcpool.tile([128, sq], f32r,
                                                  name=f"fc{bb}_{g2}_{t2}",
                                                  tag=f"fc{bb}_{g2}_{t2}")
                                if cdt == "bf16":
                                    nc.sync.dma_start(
                                        fct[:],
                                        a2a_out[bb * GROUPS + g2, t2, :, :])
                                else:
                                    nc.gpsimd.dma_start(
                                        fct[:],
                                        a2a_out[bb * GROUPS + g2, t2, :, :])
                                fc_sb.append(fct)
                        for mt in range(sq // 128):
                            ob = opool.tile([128, d], f32, name="ob", tag="ob")
                            for nn in range(d // OW):
                                ns_ = slice(nn * OW, (nn + 1) * OW)
                                ps = pso.tile([128, OW], f32, name="ps_o",
                                              tag="ps_o")
                                for kt in range(len(fc_sb)):
                                    nc.tensor.matmul(
                                        ps[:],
                                        lhsT=r(fc_sb[kt][:, mt * 128:(mt + 1) * 128]),
                                        rhs=r(wo_sb[kt][:, ns_]),
                                        start=(kt == 0),
                                        stop=(kt == len(fc_sb) - 1),
                                    )
                                nc.vector.tensor_add(ob[:, ns_], ps[:],
                                                     bob_sb[:, ns_])
                            nc.sync.dma_start(
                                outx_d[mt * 128:(mt + 1) * 128, :], ob[:])
            else:
                # ------- Phase 4 (host mode): partial fc_out, full seq ------
                with tc.tile_pool(name="osb", bufs=3) as opool, \
                     tc.tile_pool(name="pso", bufs=4, space="PSUM") as pso:
                    for mt in range(NS):
                        ob = opool.tile([128, d], f32, name="ob", tag="ob")
                        for nn in range(d // OW):
                            ns_ = slice(nn * OW, (nn + 1) * OW)
                            ps = pso.tile([128, OW], f32, name="ps_o",
                                          tag="ps_o")
                            for kt in range(MC):
                                nc.tensor.matmul(
                                    ps[:],
                                    lhsT=r(attn_sb[kt][:, mt * 128:(mt + 1) * 128]),
                                    rhs=r(wo_sb_g(wo_sb, kt)[:, ns_]),
                                    start=(kt == 0), stop=(kt == MC - 1),
                                )
                            nc.vector.tensor_add(ob[:, ns_], ps[:],
                                                 bob_sb[:, ns_])
                        nc.sync.dma_start(out_d[mt * 128:(mt + 1) * 128, :],
                                          ob[:])

    nc.compile()
    return nc


def wo_sb_g(wo_sb, kt):
    # host mode: contraction is only over this core's C rows of Wo; the host
    # passes the [C, d] slice in "wo" (padded tile list indexed 0..MC-1)
    return wo_sb[kt]


def make_in_maps(x, Wq, bq, Wk, bk, Wv, bv, Wo, bo, n_cores=N_CORES, mode=MODE,
                 cdt=None):
    cdt = cdt or CDT
    d = x.shape[2]
    MC = C // 128
    f = np.float32
    if cdt == "bf16":
        import ml_dtypes
        cf = ml_dtypes.bfloat16
    else:
        cf = np.float32
    in_maps = []
    for core in range(n_cores):
        b, g = divmod(core, GROUPS)
        cs = slice(g * C, (g + 1) * C)
        m = {
            "xT": np.ascontiguousarray(x[b].T.astype(cf)),
            "wq": np.ascontiguousarray(Wq[:, cs].astype(cf)),
            "wk": np.ascontiguousarray(Wk[:, cs].astype(cf)),
            "wv": np.ascontiguousarray(Wv[:, cs].astype(cf)),
            "bqs": np.ascontiguousarray(bq[cs].reshape(MC, 128).T, dtype=f),
            "bks": np.ascontiguousarray(bk[cs].reshape(MC, 128).T, dtype=f),
            "bvb": np.ascontiguousarray(np.broadcast_to(bv[cs], (128, C)), dtype=f),
        }
        if mode == "a2a":
            m["wo"] = np.ascontiguousarray(Wo.astype(cf))
            m["bob"] = np.ascontiguousarray(np.broadcast_to(bo, (128, d)), dtype=f)
        else:
            m["wo"] = np.ascontiguousarray(Wo[cs].astype(cf))
            bob = np.broadcast_to(bo, (128, d)).astype(f) if g == 0 else \
                np.zeros((128, d), f)
            m["bob"] = np.ascontiguousarray(bob)
        in_maps.append(m)
    return in_maps


_nc_cache = {}


def _get_nc(mode=MODE):
    key = ("nc", mode)
    if key not in _nc_cache:
        _nc_cache[key] = build_nc(mode=mode)
    return _nc_cache[key]


def assemble(results, mode=MODE):
    out = np.empty((B, S, D), np.float32)
    if mode == "a2a":
        for core in range(N_CORES):
            b, g = divmod(core, GROUPS)
            out[b, g * SQ:(g + 1) * SQ, :] = results[core][f"out{b}"]
    else:
        for b in range(B):
            acc = results[b * GROUPS]["out"].copy()
            for g in range(1, GROUPS):
                acc += results[b * GROUPS + g]["out"]
            out[b] = acc
    return out


def kernel(x, Wq, bq, Wk, bk, Wv, bv, Wo, bo, _trace=False, _mode=None):
    from concourse.bass_utils import run_bass_kernel_spmd

    mode = _mode or MODE
    nc = _get_nc(mode)
    in_maps = make_in_maps(x, Wq, bq, Wk, bk, Wv, bv, Wo, bo, mode=mode)
    res = run_bass_kernel_spmd(nc, in_maps, core_ids=list(range(N_CORES)),
                               trace=_trace)
    _nc_cache["last_result"] = res
    return assemble(res.results, mode=mode)
